# revision 6
# baseline (speedup 1.0000x reference)
"""Expert-parallel batched FFN kernel for Trainium2 (8 NeuronCores).

Problem: y[e] = relu(x[e] @ fc1_w[e] + fc1_b[e]) @ fc2_w[e] + fc2_b[e]
with E=4, T=4096, D=2048, H=8192, fp32.

Sharding: expert-parallel × token-parallel. Core c handles expert e=c//2 and
token half h=c%2 (2048 tokens). Each core holds its expert's full weights, so
no collectives are needed.

Layout trick: both matmuls keep the contraction dim on SBUF partitions by
computing everything transposed:
    y1T[h,t] = W1[d,h].T-contract  (lhsT = W1 natural, rhs = xT)
    outT[d,t] = W2[h,d].T-contract (lhsT = W2 natural, rhs = y1T)
The host passes x pre-transposed (xT) and transposes the returned outT back,
so the device never transposes anything. Matmuls run as float32r (full PE
rate for moving dim >= 256). Layer-2 accumulation over all of H is done in
SBUF via DVE adds of per-h-chunk PSUM partials.
"""

import sys

for _p in ("/opt/trn_rl_repo", "/root/.axon_site/_ro/trn_rl_repo"):
    if _p not in sys.path:
        sys.path.append(_p)

import numpy as np

import concourse.bass as bass  # noqa: F401  (registers types)
import concourse.mybir as mybir
import concourse.tile as tile
from concourse import bacc
from concourse.bass_utils import run_bass_kernel_spmd

# Problem shapes (hardcoded per contract)
E, T, D, H = 4, 4096, 2048, 8192
NCORES = 8
TL = T * E // NCORES  # 2048 tokens per core
P = 128

# Tiling
TB = 512          # moving-dim (token) block per matmul
TPW = 1024        # tokens per weight pass (t-pair)
NTP = TL // TPW   # 2 weight passes
HC = 256          # h-chunk columns
NHC = H // HC     # 32 h-chunks
DK = D // P       # 16 k-subtiles for layer 1
HK = HC // P      # 2 k-subtiles per h-chunk for layer 2
DM = D // P       # 16 d-subtiles of the output

F32 = mybir.dt.float32
F32R = mybir.dt.float32r


def _build():
    nc = bacc.Bacc("TRN2", target_bir_lowering=False, debug=False,
                   num_devices=NCORES)

    xT = nc.dram_tensor("xT", [D, TL], F32, kind="ExternalInput")
    w1 = nc.dram_tensor("w1", [D, H], F32, kind="ExternalInput")
    b1 = nc.dram_tensor("b1", [H], F32, kind="ExternalInput")
    w2 = nc.dram_tensor("w2", [H, D], F32, kind="ExternalInput")
    b2 = nc.dram_tensor("b2", [D], F32, kind="ExternalInput")
    outT = nc.dram_tensor("outT", [D, TL], F32, kind="ExternalOutput")

    xT_r = xT[:].rearrange("(k p) t -> p k t", p=P).bitcast(F32R)    # [128,16,TL]
    w1_r = w1[:].rearrange("(k p) h -> p k h", p=P).bitcast(F32R)    # [128,16,H]
    w2_r = w2[:].rearrange("(k p) d -> p k d", p=P).bitcast(F32R)    # [128,64,D]
    outT_r = outT[:].rearrange("(s p) t -> p s t", p=P)              # [128,16,TL]
    b1_r = b1[:].rearrange("(s p) -> p s", p=P)                      # [128,64]
    b2_r = b2[:].rearrange("(s p) -> p s", p=P)                      # [128,16]

    with tile.TileContext(nc) as tc:
        with (
            tc.tile_pool(name="consts", bufs=1) as cpool,
            tc.tile_pool(name="xp", bufs=DK + 1) as xpool,
            tc.tile_pool(name="accp", bufs=DM) as accpool,
            tc.tile_pool(name="w1p", bufs=2) as w1pool,
            tc.tile_pool(name="w2p", bufs=2) as w2pool,
            tc.tile_pool(name="y1p", bufs=2) as y1pool,
            tc.tile_pool(name="ps1", bufs=3, space="PSUM") as ps1pool,
            tc.tile_pool(name="ps2", bufs=4, space="PSUM") as ps2pool,
        ):
            b1_sb = cpool.tile([P, H // P], F32)
            b2_sb = cpool.tile([P, D // P], F32)
            # gpsimd queue: keeps these descriptor-heavy scatters off the
            # sync queues that feed the first matmul chain.
            nc.gpsimd.dma_start(b1_sb[:], b1_r)
            nc.gpsimd.dma_start(b2_sb[:], b2_r)

            for tp in range(NTP):
                t0 = tp * TPW
                # Per-k xT tiles, loaded per (k, tb-half) and interleaved
                # with the first h-chunk's W1 slices, so the very first
                # matmul chain is gated on ~384KB, not on the whole 8MB.
                # Extra pool slots let the next pair's first slices prefetch
                # before this pair finishes.
                xs = [
                    xpool.tile([P, TPW], F32R, tag="xk", name=f"xk{k}")
                    for k in range(DK)
                ]
                w1c0 = w1pool.tile([P, DK, HC], F32R, tag="w1c")
                for k in range(DK):
                    nc.sync.dma_start(w1c0[:, k, :], w1_r[:, k, 0:HC])
                    nc.sync.dma_start(xs[k][:, 0:TB], xT_r[:, k, t0:t0 + TB])
                w2c0 = w2pool.tile([P, HK, D], F32R, tag="w2c")
                for k in range(HK):
                    nc.sync.dma_start(w2c0[:, k, :], w2_r[:, k, :])
                for k in range(DK):
                    nc.sync.dma_start(
                        xs[k][:, TB:TPW], xT_r[:, k, t0 + TB:t0 + TPW]
                    )

                # Per-s output accumulators (bias-initialised); per-s slot
                # recycling lets drains/inits pipeline across pairs.
                accs = []
                for s in range(DM):
                    a_s = accpool.tile([P, TPW], F32, tag="accs")
                    nc.vector.tensor_copy(
                        a_s[:], b2_sb[:, s, None].to_broadcast((P, TPW))
                    )
                    accs.append(a_s)

                for hc in range(NHC):
                    if hc == 0:
                        w1c, w2c = w1c0, w2c0
                    else:
                        w1c = w1pool.tile([P, DK, HC], F32R, tag="w1c")
                        for k in range(DK):
                            nc.sync.dma_start(
                                w1c[:, k, :], w1_r[:, k, hc * HC:(hc + 1) * HC]
                            )
                        w2c = w2pool.tile([P, HK, D], F32R, tag="w2c")
                        for k in range(HK):
                            nc.sync.dma_start(
                                w2c[:, k, :], w2_r[:, hc * HK + k, :]
                            )

                    for tb in range(TPW // TB):
                        tsl = slice(tb * TB, (tb + 1) * TB)
                        y1t = y1pool.tile([P, HK, TB], F32R)
                        # Layer 1: y1T chunk = relu(W1c.T @ xT + b1)
                        for m in range(HK):
                            ps = ps1pool.tile([P, TB], F32)
                            for k in range(DK):
                                nc.tensor.matmul(
                                    ps[:],
                                    w1c[:, k, m * P:(m + 1) * P],
                                    xs[k][:, tsl],
                                    start=(k == 0),
                                    stop=(k == DK - 1),
                                )
                            nc.scalar.activation(
                                y1t[:, m, :], ps[:],
                                mybir.ActivationFunctionType.Relu,
                                bias=b1_sb[:, hc * HK + m:hc * HK + m + 1],
                            )
                        # Layer 2: acc += W2c.T @ y1t
                        for n in range(DM):
                            ps = ps2pool.tile([P, TB], F32)
                            for k in range(HK):
                                nc.tensor.matmul(
                                    ps[:],
                                    w2c[:, k, n * P:(n + 1) * P],
                                    y1t[:, k, :],
                                    start=(k == 0),
                                    stop=(k == HK - 1),
                                )
                            nc.vector.tensor_add(
                                accs[n][:, tsl], accs[n][:, tsl], ps[:]
                            )

                for s in range(DM):
                    nc.sync.dma_start(outT_r[:, s, t0:t0 + TPW], accs[s][:])

    nc.compile()
    return nc


_NC_CACHE = None


def _get_nc():
    global _NC_CACHE
    if _NC_CACHE is None:
        _NC_CACHE = _build()
    return _NC_CACHE


def _make_in_maps(x, fc1_w, fc1_b, fc2_w, fc2_b):
    in_maps = []
    for c in range(NCORES):
        e, th = divmod(c, NCORES // E)
        xs = x[e, th * TL:(th + 1) * TL, :]  # (TL, D)
        in_maps.append({
            "xT": np.ascontiguousarray(xs.T),
            "w1": np.ascontiguousarray(fc1_w[e]),
            "b1": np.ascontiguousarray(fc1_b[e, 0, :]),
            "w2": np.ascontiguousarray(fc2_w[e]),
            "b2": np.ascontiguousarray(fc2_b[e, 0, :]),
        })
    return in_maps


def run_spmd(in_maps, trace=False, **kwargs):
    """Compile (cached) and run the SPMD kernel; returns BassKernelResults."""
    nc = _get_nc()
    return run_bass_kernel_spmd(nc, in_maps, core_ids=list(range(NCORES)),
                                trace=trace, **kwargs)


def kernel(x, fc1_w, fc1_b, fc2_w, fc2_b):
    x = np.asarray(x, dtype=np.float32)
    fc1_w = np.asarray(fc1_w, dtype=np.float32)
    fc1_b = np.asarray(fc1_b, dtype=np.float32)
    fc2_w = np.asarray(fc2_w, dtype=np.float32)
    fc2_b = np.asarray(fc2_b, dtype=np.float32)

    in_maps = _make_in_maps(x, fc1_w, fc1_b, fc2_w, fc2_b)
    res = run_spmd(in_maps)

    out = np.empty((E, T, D), dtype=np.float32)
    for c in range(NCORES):
        e, th = divmod(c, NCORES // E)
        out[e, th * TL:(th + 1) * TL, :] = res.results[c]["outT"].T
    return out


# revision 7
# speedup vs baseline: 1.0101x; 1.0101x over previous
"""Expert-parallel batched FFN kernel for Trainium2 (8 NeuronCores).

Problem: y[e] = relu(x[e] @ fc1_w[e] + fc1_b[e]) @ fc2_w[e] + fc2_b[e]
with E=4, T=4096, D=2048, H=8192, fp32.

Sharding: expert-parallel × token-parallel. Core c handles expert e=c//2 and
token half h=c%2 (2048 tokens). Each core holds its expert's full weights, so
no collectives are needed.

Layout trick: both matmuls keep the contraction dim on SBUF partitions by
computing everything transposed:
    y1T[h,t] = W1[d,h].T-contract  (lhsT = W1 natural, rhs = xT)
    outT[d,t] = W2[h,d].T-contract (lhsT = W2 natural, rhs = y1T)
The host passes x pre-transposed (xT) and transposes the returned outT back,
so the device never transposes anything. Matmuls run as float32r (full PE
rate for moving dim >= 256). Layer-2 accumulation over all of H is done in
SBUF via DVE adds of per-h-chunk PSUM partials.
"""

import sys

for _p in ("/opt/trn_rl_repo", "/root/.axon_site/_ro/trn_rl_repo"):
    if _p not in sys.path:
        sys.path.append(_p)

import numpy as np

import concourse.bass as bass  # noqa: F401  (registers types)
import concourse.mybir as mybir
import concourse.tile as tile
from concourse import bacc
from concourse.bass_utils import run_bass_kernel_spmd

# Problem shapes (hardcoded per contract)
E, T, D, H = 4, 4096, 2048, 8192
NCORES = 8
TL = T * E // NCORES  # 2048 tokens per core
P = 128

# Tiling
TB = 512          # moving-dim (token) block per matmul
TPW = 1024        # tokens per weight pass (t-pair)
NTP = TL // TPW   # 2 weight passes
HC = 256          # h-chunk columns
NHC = H // HC     # 32 h-chunks
DK = D // P       # 16 k-subtiles for layer 1
HK = HC // P      # 2 k-subtiles per h-chunk for layer 2
DM = D // P       # 16 d-subtiles of the output

F32 = mybir.dt.float32
F32R = mybir.dt.float32r


def _build():
    nc = bacc.Bacc("TRN2", target_bir_lowering=False, debug=False,
                   num_devices=NCORES)

    xT = nc.dram_tensor("xT", [D, TL], F32, kind="ExternalInput")
    w1 = nc.dram_tensor("w1", [D, H], F32, kind="ExternalInput")
    b1 = nc.dram_tensor("b1", [H], F32, kind="ExternalInput")
    w2 = nc.dram_tensor("w2", [H, D], F32, kind="ExternalInput")
    b2 = nc.dram_tensor("b2", [D], F32, kind="ExternalInput")
    outT = nc.dram_tensor("outT", [D, TL], F32, kind="ExternalOutput")

    xT_r = xT[:].rearrange("(k p) t -> p k t", p=P).bitcast(F32R)    # [128,16,TL]
    w1_r = w1[:].rearrange("(k p) h -> p k h", p=P).bitcast(F32R)    # [128,16,H]
    w2_r = w2[:].rearrange("(k p) d -> p k d", p=P).bitcast(F32R)    # [128,64,D]
    outT_r = outT[:].rearrange("(s p) t -> p s t", p=P)              # [128,16,TL]
    b1_r = b1[:].rearrange("(s p) -> p s", p=P)                      # [128,64]
    b2_r = b2[:].rearrange("(s p) -> p s", p=P)                      # [128,16]

    with tile.TileContext(nc) as tc:
        with (
            tc.tile_pool(name="consts", bufs=1) as cpool,
            tc.tile_pool(name="xp", bufs=DK + 1) as xpool,
            tc.tile_pool(name="accp", bufs=DM) as accpool,
            tc.tile_pool(name="w1p", bufs=2) as w1pool,
            tc.tile_pool(name="w2p", bufs=2) as w2pool,
            tc.tile_pool(name="y1p", bufs=2) as y1pool,
            tc.tile_pool(name="ps1", bufs=3, space="PSUM") as ps1pool,
            tc.tile_pool(name="ps2", bufs=4, space="PSUM") as ps2pool,
        ):
            b1_sb = cpool.tile([P, H // P], F32)
            b2_sb = cpool.tile([P, D // P], F32)
            # gpsimd queue: keeps these descriptor-heavy scatters off the
            # sync queues that feed the first matmul chain.
            nc.gpsimd.dma_start(b1_sb[:], b1_r)
            nc.gpsimd.dma_start(b2_sb[:], b2_r)

            for tp in range(NTP):
                t0 = tp * TPW
                # Per-k xT tiles, loaded per (k, tb-half) and interleaved
                # with the first h-chunk's W1 slices, so the very first
                # matmul chain is gated on ~384KB, not on the whole 8MB.
                # Extra pool slots let the next pair's first slices prefetch
                # before this pair finishes.
                xs = [
                    xpool.tile([P, TPW], F32R, tag="xk", name=f"xk{k}")
                    for k in range(DK)
                ]
                w1c0 = w1pool.tile([P, DK, HC], F32R, tag="w1c")
                for k in range(DK):
                    nc.sync.dma_start(w1c0[:, k, :], w1_r[:, k, 0:HC])
                    nc.sync.dma_start(xs[k][:, 0:TB], xT_r[:, k, t0:t0 + TB])
                w2c0 = w2pool.tile([P, HK, D], F32R, tag="w2c")
                for k in range(HK):
                    nc.sync.dma_start(w2c0[:, k, :], w2_r[:, k, :])
                for k in range(DK):
                    nc.sync.dma_start(
                        xs[k][:, TB:TPW], xT_r[:, k, t0 + TB:t0 + TPW]
                    )

                # Per-s output accumulators (bias-initialised); per-s slot
                # recycling lets drains/inits pipeline across pairs.
                accs = []
                for s in range(DM):
                    a_s = accpool.tile([P, TPW], F32, tag="accs")
                    nc.vector.tensor_copy(
                        a_s[:], b2_sb[:, s, None].to_broadcast((P, TPW))
                    )
                    accs.append(a_s)

                for hc in range(NHC):
                    if hc == 0:
                        w1c, w2c = w1c0, w2c0
                    else:
                        w1c = w1pool.tile([P, DK, HC], F32R, tag="w1c")
                        for k in range(DK):
                            nc.sync.dma_start(
                                w1c[:, k, :], w1_r[:, k, hc * HC:(hc + 1) * HC]
                            )
                        w2c = w2pool.tile([P, HK, D], F32R, tag="w2c")
                        for k in range(HK):
                            nc.sync.dma_start(
                                w2c[:, k, :], w2_r[:, hc * HK + k, :]
                            )

                    # k-descending for the first h-chunk: its first matmul
                    # then reads the LAST-arriving input slice, so the chain
                    # only starts once everything is resident and the PE
                    # runs dense from the first instruction (no HAM thrash
                    # from DMA-paced stuttering).
                    korder = (
                        list(range(DK - 1, -1, -1)) if hc == 0
                        else list(range(DK))
                    )
                    last = hc == NHC - 1

                    def layer1(tb, y1t):
                        tsl = slice(tb * TB, (tb + 1) * TB)
                        for m in range(HK):
                            ps = ps1pool.tile([P, TB], F32, name="ps1t")
                            for j, k in enumerate(korder):
                                nc.tensor.matmul(
                                    ps[:],
                                    w1c[:, k, m * P:(m + 1) * P],
                                    xs[k][:, tsl],
                                    start=(j == 0),
                                    stop=(j == DK - 1),
                                )
                            nc.scalar.activation(
                                y1t[:, m, :], ps[:],
                                mybir.ActivationFunctionType.Relu,
                                bias=b1_sb[:, hc * HK + m:hc * HK + m + 1],
                            )

                    def layer2(tb, y1t):
                        tsl = slice(tb * TB, (tb + 1) * TB)
                        for n in range(DM):
                            ps = ps2pool.tile([P, TB], F32, name="ps2t")
                            for k in range(HK):
                                nc.tensor.matmul(
                                    ps[:],
                                    w2c[:, k, n * P:(n + 1) * P],
                                    y1t[:, k, :],
                                    start=(k == 0),
                                    stop=(k == HK - 1),
                                )
                            nc.vector.tensor_add(
                                accs[n][:, tsl], accs[n][:, tsl], ps[:]
                            )

                    y1ts = [
                        y1pool.tile([P, HK, TB], F32R, tag="y1t",
                                    name=f"y1t{tb}")
                        for tb in range(TPW // TB)
                    ]
                    if last:
                        # All L1 (x reads) first, then the x-free L2 tail —
                        # the next pair's x slices load during the L2 span.
                        for tb in range(TPW // TB):
                            layer1(tb, y1ts[tb])
                        for tb in range(TPW // TB):
                            layer2(tb, y1ts[tb])
                    else:
                        for tb in range(TPW // TB):
                            layer1(tb, y1ts[tb])
                            layer2(tb, y1ts[tb])

                for s in range(DM):
                    nc.sync.dma_start(outT_r[:, s, t0:t0 + TPW], accs[s][:])

    nc.compile()
    return nc


_NC_CACHE = None


def _get_nc():
    global _NC_CACHE
    if _NC_CACHE is None:
        _NC_CACHE = _build()
    return _NC_CACHE


def _make_in_maps(x, fc1_w, fc1_b, fc2_w, fc2_b):
    in_maps = []
    for c in range(NCORES):
        e, th = divmod(c, NCORES // E)
        xs = x[e, th * TL:(th + 1) * TL, :]  # (TL, D)
        in_maps.append({
            "xT": np.ascontiguousarray(xs.T),
            "w1": np.ascontiguousarray(fc1_w[e]),
            "b1": np.ascontiguousarray(fc1_b[e, 0, :]),
            "w2": np.ascontiguousarray(fc2_w[e]),
            "b2": np.ascontiguousarray(fc2_b[e, 0, :]),
        })
    return in_maps


def run_spmd(in_maps, trace=False, **kwargs):
    """Compile (cached) and run the SPMD kernel; returns BassKernelResults."""
    nc = _get_nc()
    return run_bass_kernel_spmd(nc, in_maps, core_ids=list(range(NCORES)),
                                trace=trace, **kwargs)


def kernel(x, fc1_w, fc1_b, fc2_w, fc2_b):
    x = np.asarray(x, dtype=np.float32)
    fc1_w = np.asarray(fc1_w, dtype=np.float32)
    fc1_b = np.asarray(fc1_b, dtype=np.float32)
    fc2_w = np.asarray(fc2_w, dtype=np.float32)
    fc2_b = np.asarray(fc2_b, dtype=np.float32)

    in_maps = _make_in_maps(x, fc1_w, fc1_b, fc2_w, fc2_b)
    res = run_spmd(in_maps)

    out = np.empty((E, T, D), dtype=np.float32)
    for c in range(NCORES):
        e, th = divmod(c, NCORES // E)
        out[e, th * TL:(th + 1) * TL, :] = res.results[c]["outT"].T
    return out


# revision 9
# speedup vs baseline: 1.0155x; 1.0054x over previous
"""Expert-parallel batched FFN kernel for Trainium2 (8 NeuronCores).

Problem: y[e] = relu(x[e] @ fc1_w[e] + fc1_b[e]) @ fc2_w[e] + fc2_b[e]
with E=4, T=4096, D=2048, H=8192, fp32.

Sharding: expert-parallel × token-parallel. Core c handles expert e=c//2 and
token half h=c%2 (2048 tokens). Each core holds its expert's full weights, so
no collectives are needed.

Layout trick: both matmuls keep the contraction dim on SBUF partitions by
computing everything transposed:
    y1T[h,t] = W1[d,h].T-contract  (lhsT = W1 natural, rhs = xT)
    outT[d,t] = W2[h,d].T-contract (lhsT = W2 natural, rhs = y1T)
The host passes x pre-transposed (xT) and transposes the returned outT back,
so the device never transposes anything. Matmuls run as float32r (full PE
rate for moving dim >= 256). Layer-2 accumulation over all of H is done in
SBUF via DVE adds of per-h-chunk PSUM partials.
"""

import sys

for _p in ("/opt/trn_rl_repo", "/root/.axon_site/_ro/trn_rl_repo"):
    if _p not in sys.path:
        sys.path.append(_p)

import numpy as np

import concourse.bass as bass  # noqa: F401  (registers types)
import concourse.mybir as mybir
import concourse.tile as tile
from concourse import bacc
from concourse.bass_utils import run_bass_kernel_spmd

# Problem shapes (hardcoded per contract)
E, T, D, H = 4, 4096, 2048, 8192
NCORES = 8
TL = T * E // NCORES  # 2048 tokens per core
P = 128

# Tiling
TB = 512          # moving-dim (token) block per matmul
TPW = 1024        # tokens per weight pass (t-pair)
NTP = TL // TPW   # 2 weight passes
HC = 256          # h-chunk columns
NHC = H // HC     # 32 h-chunks
DK = D // P       # 16 k-subtiles for layer 1
HK = HC // P      # 2 k-subtiles per h-chunk for layer 2
DM = D // P       # 16 d-subtiles of the output

F32 = mybir.dt.float32
F32R = mybir.dt.float32r


def _build():
    nc = bacc.Bacc("TRN2", target_bir_lowering=False, debug=False,
                   num_devices=NCORES)

    xT = nc.dram_tensor("xT", [D, TL], F32, kind="ExternalInput")
    w1 = nc.dram_tensor("w1", [D, H], F32, kind="ExternalInput")
    b1 = nc.dram_tensor("b1", [H], F32, kind="ExternalInput")
    w2 = nc.dram_tensor("w2", [H, D], F32, kind="ExternalInput")
    b2 = nc.dram_tensor("b2", [D], F32, kind="ExternalInput")
    outT = nc.dram_tensor("outT", [D, TL], F32, kind="ExternalOutput")

    xT_r = xT[:].rearrange("(k p) t -> p k t", p=P).bitcast(F32R)    # [128,16,TL]
    w1_r = w1[:].rearrange("(k p) h -> p k h", p=P).bitcast(F32R)    # [128,16,H]
    w2_r = w2[:].rearrange("(k p) d -> p k d", p=P).bitcast(F32R)    # [128,64,D]
    outT_r = outT[:].rearrange("(s p) t -> p s t", p=P)              # [128,16,TL]
    b1_r = b1[:].rearrange("(s p) -> p s", p=P)                      # [128,64]
    b2_r = b2[:].rearrange("(s p) -> p s", p=P)                      # [128,16]

    with tile.TileContext(nc) as tc:
        with (
            tc.tile_pool(name="consts", bufs=1) as cpool,
            tc.tile_pool(name="xp", bufs=DK + 1) as xpool,
            tc.tile_pool(name="accp", bufs=DM) as accpool,
            tc.tile_pool(name="w1p", bufs=2) as w1pool,
            tc.tile_pool(name="w2p", bufs=2) as w2pool,
            tc.tile_pool(name="y1p", bufs=2) as y1pool,
            tc.tile_pool(name="ps1", bufs=3, space="PSUM") as ps1pool,
            tc.tile_pool(name="ps2", bufs=4, space="PSUM") as ps2pool,
        ):
            b1_sb = cpool.tile([P, H // P], F32)
            b2_sb = cpool.tile([P, D // P], F32)
            # gpsimd queue: keeps these descriptor-heavy scatters off the
            # sync queues that feed the first matmul chain.
            nc.gpsimd.dma_start(b1_sb[:], b1_r)
            nc.gpsimd.dma_start(b2_sb[:], b2_r)

            for tp in range(NTP):
                t0 = tp * TPW
                # Per-k xT tiles, loaded per (k, tb-half) and interleaved
                # with the first h-chunk's W1 slices, so the very first
                # matmul chain is gated on ~384KB, not on the whole 8MB.
                # Extra pool slots let the next pair's first slices prefetch
                # before this pair finishes.
                xs = [
                    xpool.tile([P, TPW], F32R, tag="xk", name=f"xk{k}")
                    for k in range(DK)
                ]
                # Priming: first h-chunk's weights go on the gpsimd (SWDGE)
                # queues so they stream concurrently with the x slices on
                # the sync (HWDGE) queues.
                w1c0 = w1pool.tile([P, DK, HC], F32R, tag="w1c")
                for k in range(DK):
                    nc.gpsimd.dma_start(w1c0[:, k, :], w1_r[:, k, 0:HC])
                    nc.sync.dma_start(xs[k][:, 0:TB], xT_r[:, k, t0:t0 + TB])
                w2c0 = w2pool.tile([P, HK, D], F32R, tag="w2c")
                for k in range(HK):
                    nc.gpsimd.dma_start(w2c0[:, k, :], w2_r[:, k, :])
                for k in range(DK):
                    nc.sync.dma_start(
                        xs[k][:, TB:TPW], xT_r[:, k, t0 + TB:t0 + TPW]
                    )

                # Per-s output accumulators (bias-initialised); per-s slot
                # recycling lets drains/inits pipeline across pairs.
                accs = []
                for s in range(DM):
                    a_s = accpool.tile([P, TPW], F32, tag="accs")
                    nc.vector.tensor_copy(
                        a_s[:], b2_sb[:, s, None].to_broadcast((P, TPW))
                    )
                    accs.append(a_s)

                for hc in range(NHC):
                    if hc == 0:
                        w1c, w2c = w1c0, w2c0
                    else:
                        w1c = w1pool.tile([P, DK, HC], F32R, tag="w1c")
                        for k in range(DK):
                            nc.sync.dma_start(
                                w1c[:, k, :], w1_r[:, k, hc * HC:(hc + 1) * HC]
                            )
                        w2c = w2pool.tile([P, HK, D], F32R, tag="w2c")
                        for k in range(HK):
                            nc.sync.dma_start(
                                w2c[:, k, :], w2_r[:, hc * HK + k, :]
                            )

                    # k-descending for the first h-chunk: its first matmul
                    # then reads the LAST-arriving input slice, so the chain
                    # only starts once everything is resident and the PE
                    # runs dense from the first instruction (no HAM thrash
                    # from DMA-paced stuttering).
                    korder = (
                        list(range(DK - 1, -1, -1)) if hc == 0
                        else list(range(DK))
                    )
                    last = hc == NHC - 1

                    def layer1(tb, y1t):
                        tsl = slice(tb * TB, (tb + 1) * TB)
                        for m in range(HK):
                            ps = ps1pool.tile([P, TB], F32, name="ps1t")
                            for j, k in enumerate(korder):
                                nc.tensor.matmul(
                                    ps[:],
                                    w1c[:, k, m * P:(m + 1) * P],
                                    xs[k][:, tsl],
                                    start=(j == 0),
                                    stop=(j == DK - 1),
                                )
                            nc.scalar.activation(
                                y1t[:, m, :], ps[:],
                                mybir.ActivationFunctionType.Relu,
                                bias=b1_sb[:, hc * HK + m:hc * HK + m + 1],
                            )

                    def layer2(tb, y1t):
                        tsl = slice(tb * TB, (tb + 1) * TB)
                        for n in range(DM):
                            ps = ps2pool.tile([P, TB], F32, name="ps2t")
                            for k in range(HK):
                                nc.tensor.matmul(
                                    ps[:],
                                    w2c[:, k, n * P:(n + 1) * P],
                                    y1t[:, k, :],
                                    start=(k == 0),
                                    stop=(k == HK - 1),
                                )
                            nc.vector.tensor_add(
                                accs[n][:, tsl], accs[n][:, tsl], ps[:]
                            )

                    y1ts = [
                        y1pool.tile([P, HK, TB], F32R, tag="y1t",
                                    name=f"y1t{tb}")
                        for tb in range(TPW // TB)
                    ]
                    if last:
                        # All L1 (x reads) first, then the x-free L2 tail —
                        # the next pair's x slices load during the L2 span.
                        for tb in range(TPW // TB):
                            layer1(tb, y1ts[tb])
                        for tb in range(TPW // TB):
                            layer2(tb, y1ts[tb])
                    else:
                        for tb in range(TPW // TB):
                            layer1(tb, y1ts[tb])
                            layer2(tb, y1ts[tb])

                # Output drains on gpsimd queues: keeps 8MB of writes out of
                # the sync queues that the next pair's input loads need.
                for s in range(DM):
                    nc.gpsimd.dma_start(outT_r[:, s, t0:t0 + TPW], accs[s][:])

    nc.compile()
    return nc


_NC_CACHE = None


def _get_nc():
    global _NC_CACHE
    if _NC_CACHE is None:
        _NC_CACHE = _build()
    return _NC_CACHE


def _make_in_maps(x, fc1_w, fc1_b, fc2_w, fc2_b):
    in_maps = []
    for c in range(NCORES):
        e, th = divmod(c, NCORES // E)
        xs = x[e, th * TL:(th + 1) * TL, :]  # (TL, D)
        in_maps.append({
            "xT": np.ascontiguousarray(xs.T),
            "w1": np.ascontiguousarray(fc1_w[e]),
            "b1": np.ascontiguousarray(fc1_b[e, 0, :]),
            "w2": np.ascontiguousarray(fc2_w[e]),
            "b2": np.ascontiguousarray(fc2_b[e, 0, :]),
        })
    return in_maps


def run_spmd(in_maps, trace=False, **kwargs):
    """Compile (cached) and run the SPMD kernel; returns BassKernelResults."""
    nc = _get_nc()
    return run_bass_kernel_spmd(nc, in_maps, core_ids=list(range(NCORES)),
                                trace=trace, **kwargs)


def kernel(x, fc1_w, fc1_b, fc2_w, fc2_b):
    x = np.asarray(x, dtype=np.float32)
    fc1_w = np.asarray(fc1_w, dtype=np.float32)
    fc1_b = np.asarray(fc1_b, dtype=np.float32)
    fc2_w = np.asarray(fc2_w, dtype=np.float32)
    fc2_b = np.asarray(fc2_b, dtype=np.float32)

    in_maps = _make_in_maps(x, fc1_w, fc1_b, fc2_w, fc2_b)
    res = run_spmd(in_maps)

    out = np.empty((E, T, D), dtype=np.float32)
    for c in range(NCORES):
        e, th = divmod(c, NCORES // E)
        out[e, th * TL:(th + 1) * TL, :] = res.results[c]["outT"].T
    return out


# revision 15
# speedup vs baseline: 1.0164x; 1.0009x over previous
"""Expert-parallel batched FFN kernel for Trainium2 (8 NeuronCores).

Problem: y[e] = relu(x[e] @ fc1_w[e] + fc1_b[e]) @ fc2_w[e] + fc2_b[e]
with E=4, T=4096, D=2048, H=8192, fp32.

Sharding: expert-parallel × token-parallel. Core c handles expert e=c//2 and
token half h=c%2 (2048 tokens). Each core holds its expert's full weights, so
no collectives are needed.

Layout trick: both matmuls keep the contraction dim on SBUF partitions by
computing everything transposed:
    y1T[h,t] = W1[d,h].T-contract  (lhsT = W1 natural, rhs = xT)
    outT[d,t] = W2[h,d].T-contract (lhsT = W2 natural, rhs = y1T)
The host passes x pre-transposed (xT) and transposes the returned outT back,
so the device never transposes anything. Matmuls run as float32r (full PE
rate for moving dim >= 256). Layer-2 accumulation over all of H is done in
SBUF via DVE adds of per-h-chunk PSUM partials.
"""

import sys

for _p in ("/opt/trn_rl_repo", "/root/.axon_site/_ro/trn_rl_repo"):
    if _p not in sys.path:
        sys.path.append(_p)

import numpy as np

import concourse.bass as bass  # noqa: F401  (registers types)
import concourse.mybir as mybir
import concourse.tile as tile
from concourse import bacc
from concourse.bass_utils import run_bass_kernel_spmd

# Problem shapes (hardcoded per contract)
E, T, D, H = 4, 4096, 2048, 8192
NCORES = 8
TL = T * E // NCORES  # 2048 tokens per core
P = 128

# Tiling
TB = 512          # moving-dim (token) block per matmul
TPW = 1024        # tokens per weight pass (t-pair)
NTP = TL // TPW   # 2 weight passes
HC = 256          # h-chunk columns
NHC = H // HC     # 32 h-chunks
DK = D // P       # 16 k-subtiles for layer 1
HK = HC // P      # 2 k-subtiles per h-chunk for layer 2
DM = D // P       # 16 d-subtiles of the output
XG = 4            # k-subtiles per x group tile
NXG = DK // XG    # 4 x group tiles

F32 = mybir.dt.float32
F32R = mybir.dt.float32r


def _build():
    nc = bacc.Bacc("TRN2", target_bir_lowering=False, debug=False,
                   num_devices=NCORES)

    xT = nc.dram_tensor("xT", [D, TL], F32, kind="ExternalInput")
    w1 = nc.dram_tensor("w1", [D, H], F32, kind="ExternalInput")
    b1 = nc.dram_tensor("b1", [H], F32, kind="ExternalInput")
    w2 = nc.dram_tensor("w2", [H, D], F32, kind="ExternalInput")
    b2 = nc.dram_tensor("b2", [D], F32, kind="ExternalInput")
    outT = nc.dram_tensor("outT", [D, TL], F32, kind="ExternalOutput")

    xT_r = xT[:].rearrange("(k p) t -> p k t", p=P).bitcast(F32R)    # [128,16,TL]
    w1_r = w1[:].rearrange("(k p) h -> p k h", p=P).bitcast(F32R)    # [128,16,H]
    w2_r = w2[:].rearrange("(k p) d -> p k d", p=P).bitcast(F32R)    # [128,64,D]
    outT_r = outT[:].rearrange("(s p) t -> p s t", p=P)              # [128,16,TL]
    b1_r = b1[:].rearrange("(s p) -> p s", p=P)                      # [128,64]
    b2_r = b2[:].rearrange("(s p) -> p s", p=P)                      # [128,16]

    with tile.TileContext(nc) as tc:
        with (
            tc.tile_pool(name="consts", bufs=1) as cpool,
            tc.tile_pool(name="xp", bufs=NXG) as xpool,
            tc.tile_pool(name="accp", bufs=DM) as accpool,
            tc.tile_pool(name="w1p", bufs=2) as w1pool,
            tc.tile_pool(name="w2p", bufs=2) as w2pool,
            tc.tile_pool(name="y1p", bufs=2) as y1pool,
            tc.tile_pool(name="ps1", bufs=3, space="PSUM") as ps1pool,
            tc.tile_pool(name="ps2", bufs=4, space="PSUM") as ps2pool,
        ):
            b1_sb = cpool.tile([P, H // P], F32)
            b2_sb = cpool.tile([P, D // P], F32)
            # gpsimd queue: keeps these descriptor-heavy scatters off the
            # sync queues that feed the first matmul chain.
            nc.gpsimd.dma_start(b1_sb[:], b1_r)
            nc.gpsimd.dma_start(b2_sb[:], b2_r)

            for tp in range(NTP):
                t0 = tp * TPW
                # x group tiles ([P, XG, TPW] each): few, large DMAs — the
                # startup is dispatch-round-trip bound, not byte bound.
                # tb0 halves load first (the first chains only need tb0);
                # slot recycling is per-group so the next pair's loads
                # overlap this pair's x-free L2 tail.
                xs = [
                    xpool.tile([P, XG, TPW], F32R, tag="xg", name=f"xg{g}")
                    for g in range(NXG)
                ]
                # Priming: first h-chunk's weights go on the gpsimd (SWDGE)
                # queues so they stream concurrently with the x slices on
                # the sync (HWDGE) queues.
                w1c0 = w1pool.tile([P, DK, HC], F32R, tag="w1c")
                for g in range(NXG):
                    nc.gpsimd.dma_start(
                        w1c0[:, g * XG:(g + 1) * XG, :],
                        w1_r[:, g * XG:(g + 1) * XG, 0:HC],
                    )
                    nc.sync.dma_start(
                        xs[g][:, :, 0:TB],
                        xT_r[:, g * XG:(g + 1) * XG, t0:t0 + TB],
                    )
                w2c0 = w2pool.tile([P, HK, D], F32R, tag="w2c")
                nc.gpsimd.dma_start(w2c0[:], w2_r[:, 0:HK, :])
                for g in range(NXG):
                    nc.sync.dma_start(
                        xs[g][:, :, TB:TPW],
                        xT_r[:, g * XG:(g + 1) * XG, t0 + TB:t0 + TPW],
                    )

                # Per-s output accumulators (bias-initialised); per-s slot
                # recycling lets drains/inits pipeline across pairs.
                accs = []
                for s in range(DM):
                    a_s = accpool.tile([P, TPW], F32, tag="accs")
                    nc.vector.tensor_copy(
                        a_s[:], b2_sb[:, s, None].to_broadcast((P, TPW))
                    )
                    accs.append(a_s)

                for hc in range(NHC):
                    if hc == 0:
                        w1c, w2c = w1c0, w2c0
                    else:
                        w1c = w1pool.tile([P, DK, HC], F32R, tag="w1c")
                        for g in range(NXG):
                            nc.sync.dma_start(
                                w1c[:, g * XG:(g + 1) * XG, :],
                                w1_r[:, g * XG:(g + 1) * XG,
                                     hc * HC:(hc + 1) * HC],
                            )
                        w2c = w2pool.tile([P, HK, D], F32R, tag="w2c")
                        nc.sync.dma_start(
                            w2c[:], w2_r[:, hc * HK:(hc + 1) * HK, :]
                        )

                    # k-descending for the first h-chunk: its first matmul
                    # then reads the LAST-arriving input slice, so the chain
                    # only starts once everything is resident and the PE
                    # runs dense from the first instruction (no HAM thrash
                    # from DMA-paced stuttering).
                    korder = (
                        list(range(DK - 1, -1, -1)) if hc == 0
                        else list(range(DK))
                    )
                    last = hc == NHC - 1

                    def layer1(tb, y1t):
                        tsl = slice(tb * TB, (tb + 1) * TB)
                        for m in range(HK):
                            ps = ps1pool.tile([P, TB], F32, name="ps1t")
                            for j, k in enumerate(korder):
                                nc.tensor.matmul(
                                    ps[:],
                                    w1c[:, k, m * P:(m + 1) * P],
                                    xs[k // XG][:, k % XG, tsl],
                                    start=(j == 0),
                                    stop=(j == DK - 1),
                                )
                            nc.scalar.activation(
                                y1t[:, m, :], ps[:],
                                mybir.ActivationFunctionType.Relu,
                                bias=b1_sb[:, hc * HK + m:hc * HK + m + 1],
                            )

                    def layer2(tb, y1t):
                        tsl = slice(tb * TB, (tb + 1) * TB)
                        for n in range(DM):
                            ps = ps2pool.tile([P, TB], F32, name="ps2t")
                            for k in range(HK):
                                nc.tensor.matmul(
                                    ps[:],
                                    w2c[:, k, n * P:(n + 1) * P],
                                    y1t[:, k, :],
                                    start=(k == 0),
                                    stop=(k == HK - 1),
                                )
                            nc.vector.tensor_add(
                                accs[n][:, tsl], accs[n][:, tsl], ps[:]
                            )

                    y1ts = [
                        y1pool.tile([P, HK, TB], F32R, tag="y1t",
                                    name=f"y1t{tb}")
                        for tb in range(TPW // TB)
                    ]
                    if last:
                        # All L1 (x reads) first, then the x-free L2 tail —
                        # the next pair's x slices load during the L2 span.
                        # L2 runs s-major across both tb halves so each
                        # accumulator finishes early and drains immediately
                        # (on the gpsimd queues, clear of the sync queues
                        # that feed the next pair's input loads).
                        for tb in range(TPW // TB):
                            layer1(tb, y1ts[tb])
                        for n in range(DM):
                            for tb in range(TPW // TB):
                                tsl = slice(tb * TB, (tb + 1) * TB)
                                ps = ps2pool.tile([P, TB], F32, name="ps2t")
                                for k in range(HK):
                                    nc.tensor.matmul(
                                        ps[:],
                                        w2c[:, k, n * P:(n + 1) * P],
                                        y1ts[tb][:, k, :],
                                        start=(k == 0),
                                        stop=(k == HK - 1),
                                    )
                                nc.vector.tensor_add(
                                    accs[n][:, tsl], accs[n][:, tsl], ps[:]
                                )
                            nc.gpsimd.dma_start(
                                outT_r[:, n, t0:t0 + TPW], accs[n][:]
                            )
                    else:
                        for tb in range(TPW // TB):
                            layer1(tb, y1ts[tb])
                            layer2(tb, y1ts[tb])

    nc.compile()
    return nc


_NC_CACHE = None


def _get_nc():
    global _NC_CACHE
    if _NC_CACHE is None:
        _NC_CACHE = _build()
    return _NC_CACHE


def _make_in_maps(x, fc1_w, fc1_b, fc2_w, fc2_b):
    in_maps = []
    for c in range(NCORES):
        e, th = divmod(c, NCORES // E)
        xs = x[e, th * TL:(th + 1) * TL, :]  # (TL, D)
        in_maps.append({
            "xT": np.ascontiguousarray(xs.T),
            "w1": np.ascontiguousarray(fc1_w[e]),
            "b1": np.ascontiguousarray(fc1_b[e, 0, :]),
            "w2": np.ascontiguousarray(fc2_w[e]),
            "b2": np.ascontiguousarray(fc2_b[e, 0, :]),
        })
    return in_maps


def run_spmd(in_maps, trace=False, **kwargs):
    """Compile (cached) and run the SPMD kernel; returns BassKernelResults."""
    nc = _get_nc()
    return run_bass_kernel_spmd(nc, in_maps, core_ids=list(range(NCORES)),
                                trace=trace, **kwargs)


def kernel(x, fc1_w, fc1_b, fc2_w, fc2_b):
    x = np.asarray(x, dtype=np.float32)
    fc1_w = np.asarray(fc1_w, dtype=np.float32)
    fc1_b = np.asarray(fc1_b, dtype=np.float32)
    fc2_w = np.asarray(fc2_w, dtype=np.float32)
    fc2_b = np.asarray(fc2_b, dtype=np.float32)

    in_maps = _make_in_maps(x, fc1_w, fc1_b, fc2_w, fc2_b)
    res = run_spmd(in_maps)

    out = np.empty((E, T, D), dtype=np.float32)
    for c in range(NCORES):
        e, th = divmod(c, NCORES // E)
        out[e, th * TL:(th + 1) * TL, :] = res.results[c]["outT"].T
    return out
